# revision 15
# baseline (speedup 1.0000x reference)
"""Single-head causal attention (B=8, T=2048, C=384, H=64) on 8 NeuronCores.

Data-parallel over batch: core b computes attention for batch element b.
Per-core pipeline (all matmuls bf16, fp32 PSUM accumulate):
  - host pre-transposes x -> xT, repacked [128, 3, 2048] so each 512-col
    t-chunk is ONE dma (dma_start costs ~625ns issue on the queue engine);
    weight dmas go on the DVE/GpSimd queues so issues run in parallel
  - PE warm-up burst of junk matmuls during the input-dma wait keeps the
    HAM activity monitor from starting the real work at half clock
  - fused q/k projection: stationary [Wq|Wk] -> psum [128, 512] with qT
    on partitions 0-63 and kT on partitions 64-127 -> qk_sb
  - dup_sb = partition-rotated copy (k on 0-63, q on 64-127) via a PE
    matmul against a constant 64-rotation permutation matrix
  - score pass runs PAIRS of key blocks (2jp, 2jp+1) concurrently via PE
    row tiling (contraction is only h=64): tile A in array rows 0-63
    (stationary k_lo from dup, moving q_lo from qk), tile B in rows
    64-127 (stationary k_hi from qk, moving q_hi from dup)
  - both halves of a pair span the same t-range, so one wide ACT exp
    instruction [128, 2, w] covers both psums (ACT has ~352 cycles fixed
    cost per instruction; exp is the bottleneck engine)
  - causal: only lower-triangle block pairs computed; the two diagonal
    128x128 blocks of each pair are masked multiplicatively (GpSimd)
  - v = x @ Wv + ones column (PE, stationary xT blocks)
  - out_i = sum_j PT_j[:,i].T @ [v_j | 1]  (PE, accumulated in PSUM; the
    ones column yields the softmax denominator in col 64)
  - out   = out[:, :64] * (1 / out[:, 64]) (DVE), DMA to DRAM f32 in
    partition-major layout (host transposes back — contiguous 1KB rows
    per partition instead of 256B)
"""

import math
import os

import numpy as np
import ml_dtypes

import concourse.bass as bass
import concourse.tile as tile
from concourse import bacc, mybir
from concourse.bass import ds, ts
from concourse.bass_utils import run_bass_kernel_spmd

F32 = mybir.dt.float32
BF16 = mybir.dt.bfloat16

B, T, C, H = 8, 2048, 384, 64
P = 128
NT = T // P          # 16 t-tiles (query blocks)
NCC = C // P         # 3 contraction chunks
NPAIR = NT // 2      # 8 row-tiled key-block pairs
CHUNK = 768          # S-pass chunk width (1.5 psum banks per row-tile)
SCALE = 1.0 / math.sqrt(float(C))

# stash of the last run's results (test.py reads exec_time_ns from here)
LAST_RESULT = None
_PROGRAM = None


def _bank_pieces(a, w):
    """Split psum column range [a, a+w) at 512-col bank boundaries."""
    cur, end = a, a + w
    while cur < end:
        nxt = min(end, (cur // 512 + 1) * 512)
        yield cur, nxt - cur
        cur = nxt


def _emit(tc: tile.TileContext, xT_d, wqk_d, wv_d, mask_d, perm_d, out_d, ctx):
    nc = tc.nc
    Exp = mybir.ActivationFunctionType.Exp

    const = ctx.enter_context(tc.tile_pool(name="const", bufs=1))
    big = ctx.enter_context(tc.tile_pool(name="big", bufs=1))
    outp = ctx.enter_context(tc.tile_pool(name="outp", bufs=2))
    ps = ctx.enter_context(tc.tile_pool(name="ps", bufs=1, space="PSUM"))

    # ---- input DMAs (parallel issue queues) ------------------------------
    # SP: xT (4 chunks); Scalar: wqk, perm; GpSimd: wv, mask (swdge)
    xt_sb = big.tile([P, NCC, T], BF16, tag="xt")
    for t4 in range(4):
        nc.sync.dma_start(
            xt_sb[:, :, ts(t4, 512)], xT_d[:, :, ts(t4, 512)]
        )
    wqk_sb = const.tile([P, NCC, P], BF16, tag="wqk")
    nc.scalar.dma_start(wqk_sb[:], wqk_d.rearrange("(c p) m -> p c m", p=P))
    perm_sb = const.tile([P, P], BF16, tag="perm")
    nc.scalar.dma_start(perm_sb[:], perm_d[:])

    wt = const.tile([P, 512], BF16, tag="wt")
    nc.gpsimd.memset(wt[:], 0.0)
    warm = const.tile([1, 8], F32, tag="warm")
    nc.gpsimd.memset(warm[:, 0:4], 0.0)
    wv_sb = const.tile([P, NCC, H], BF16, tag="wv")
    nc.gpsimd.dma_start(wv_sb[:], wv_d.rearrange("(c p) h -> p c h", p=P))
    mask_sb = const.tile([P, P], BF16, tag="mask")
    nc.gpsimd.dma_start(mask_sb[:], mask_d[:])

    # ACT exp-table preload (hidden behind the input dmas)
    nc.scalar.activation(warm[:, 4:8], warm[:, 0:4], Exp, scale=1.0)

    # ---- PE warm-up: junk matmuls while input dmas are in flight ---------
    # Keeps the HAM activity window busy so real matmuls start at 2.4 GHz.
    pwarm = ps.tile([P, 1536], F32, tag="st", bufs=2, name="pwarm")
    for k in range(6):
        nc.tensor.matmul(pwarm[:, 0:512], wt[:, 0:P], wt[:],
                         start=True, stop=True)

    # ---- fused q/k projection + permuted duplicate -----------------------
    qk_sb = big.tile([P, T], BF16, tag="qk")
    dup_sb = big.tile([P, T], BF16, tag="dup")

    def proj_step(t4):
        pq = ps.tile([P, 1536], F32, tag="st", bufs=2, name=f"pqk{t4}")
        for c in range(NCC):
            nc.tensor.matmul(
                pq[:, 0:512], wqk_sb[:, c, :], xt_sb[:, c, ts(t4, 512)],
                start=(c == 0), stop=(c == NCC - 1),
            )
        nc.vector.tensor_copy(qk_sb[:, ts(t4, 512)], pq[:, 0:512])

    def perm_step(t4):
        pd = ps.tile([P, 1536], F32, tag="st", bufs=2, name=f"pdup{t4}")
        nc.tensor.matmul(pd[:, 0:512], perm_sb[:], qk_sb[:, ts(t4, 512)],
                         start=True, stop=True)
        nc.vector.tensor_copy(dup_sb[:, ts(t4, 512)], pd[:, 0:512])

    # ---- score pass: pairs of key blocks via PE row tiling ----------------
    pt_tiles = {}

    def S_gen(jp, chunks=None):
        """Generator: emits one psum chunk (both row tiles) + exp per step."""
        t0 = 2 * P * jp
        W = T - t0
        pt = big.tile([P, 2, W], BF16, tag=f"pt{jp}", name=f"pt{jp}")
        pt_tiles[jp] = (pt, t0)
        kA = dup_sb[0:64, ds(t0, P)]          # key block 2jp   (rows 0-63)
        kB = qk_sb[64:128, ds(t0 + P, P)]     # key block 2jp+1 (rows 64-127)
        qA = qk_sb[0:64, :]
        qB = dup_sb[64:128, :]
        if chunks is None:
            chunks = [min(CHUNK, W - o) for o in range(0, W, CHUNK)]
        assert sum(chunks) == W and all(c <= CHUNK for c in chunks)
        offs = [sum(chunks[:ci]) for ci in range(len(chunks))]
        for off, w in zip(offs, chunks):
            st = ps.tile([P, 1536], F32, tag="st", bufs=2, name=f"st{jp}_{off}")
            stv = st[:].rearrange("p (k m) -> p k m", k=2)
            # zip A/B bank pieces: concurrent row tiles never share a bank
            pieces_a = [(kA, qA, 0, pc, n) for pc, n in _bank_pieces(0, w)]
            pieces_b = [(kB, qB, CHUNK, pc, n)
                        for pc, n in _bank_pieces(CHUNK, w)]
            zipped = []
            for pa, pb in zip(pieces_a, pieces_b):
                zipped += [pa, pb]
            zipped += pieces_a[len(pieces_b):] + pieces_b[len(pieces_a):]
            for lhs, q_src, po, pc, n in zipped:
                nc.tensor.matmul(
                    st[:, ds(pc, n)], lhs,
                    q_src[:, ds(t0 + off + (pc - po), n)],
                    start=True, stop=True,
                )
            nc.scalar.activation(
                pt[:, :, ds(off, w)], stv[:, :, 0:w], Exp, scale=SCALE
            )
            if off == 0:
                # mask the two diagonal blocks (strict-lower of each -> 0)
                nc.gpsimd.tensor_mul(pt[:, 0, 0:P], pt[:, 0, 0:P], mask_sb[:])
                nc.gpsimd.tensor_mul(
                    pt[:, 1, P : 2 * P], pt[:, 1, P : 2 * P], mask_sb[:]
                )
            yield

    # pair 0 woven into projection + permute so the first exp fires early;
    # chunk boundaries chosen so chunk c only needs projected cols <= 512c
    g0 = S_gen(0, chunks=[256, 768, 512, 512])
    proj_step(0)
    proj_step(1)
    perm_step(0)
    next(g0, None)      # cols    0:256
    proj_step(2)
    perm_step(1)
    next(g0, None)      # cols  256:1024
    proj_step(3)
    perm_step(2)
    next(g0, None)      # cols 1024:1536
    perm_step(3)
    next(g0, None)      # cols 1536:2048
    for _ in g0:
        pass

    # ---- v projection (+ ones column for the softmax denominator) --------
    v_sb = big.tile([P, NT, 66], BF16, tag="v")
    for pk in range(2):
        pv = ps.tile([P, 1536], F32, tag="st", bufs=2, name=f"pv{pk}")
        for jj in range(8):
            j = 8 * pk + jj
            for c in range(NCC):
                nc.tensor.matmul(
                    pv[:, ts(jj, H)], xt_sb[:, c, ds(P * j, P)],
                    wv_sb[:, c, :],
                    start=(c == 0), stop=(c == NCC - 1),
                )
        nc.vector.tensor_copy(
            v_sb[:, ds(8 * pk, 8), 0:H],
            pv[:, 0:512].rearrange("p (j h) -> p j h", h=H),
        )
    nc.gpsimd.memset(v_sb[:, :, H:65], 1.0)

    # ---- output pass O(i): PV accumulate, normalize, store ----------------
    # out DRAM layout [4, 128, 4, 64]: per-partition-contiguous rows so the
    # dma moves 1KB descriptors instead of 256B; host untransposes.
    ob_ref = [None]

    def pv_out(i):
        if i % 4 == 0:
            ob_ref[0] = outp.tile([P, 4, H], F32, tag="osb", bufs=2,
                                  name=f"ob{i // 4}")
        ob = ob_ref[0]
        oa = ps.tile([P, 72], F32, tag="oacc", bufs=2, name=f"oacc{i}")
        for j in range(i + 1):
            pt, t0 = pt_tiles[j // 2]
            nc.tensor.matmul(
                oa[:, 0:65], pt[:, j % 2, ds(P * i - t0, P)],
                v_sb[:, j, 0:65],
                start=(j == 0), stop=(j == i),
            )
        r = outp.tile([P, 1], F32, tag="recip", bufs=2, name=f"r{i}")
        nc.vector.reciprocal(r[:], oa[:, 64:65])
        nc.vector.tensor_scalar_mul(ob[:, i % 4, :], oa[:, 0:H], r[:])
        if i % 4 == 3:
            nc.sync.dma_start(out_d[i // 4], ob[:])

    # weave: keep S one pair ahead of PV, with PV filling PE stalls
    for jp in range(1, NPAIR):
        g = S_gen(jp)
        next(g, None)            # chunk 0 (+ masks)
        pv_out(2 * jp - 2)
        next(g, None)            # chunk 1 (if any)
        pv_out(2 * jp - 1)
        for _ in g:              # remaining chunks
            pass
    pv_out(2 * NPAIR - 2)
    pv_out(2 * NPAIR - 1)


def _build_program():
    nc = bacc.Bacc("TRN2", target_bir_lowering=False, debug=False, num_devices=B)
    xT_d = nc.dram_tensor("xTp", [P, NCC, T], BF16, kind="ExternalInput").ap()
    wqk_d = nc.dram_tensor("wqk", [C, P], BF16, kind="ExternalInput").ap()
    wv_d = nc.dram_tensor("wv", [C, H], BF16, kind="ExternalInput").ap()
    mask_d = nc.dram_tensor("mask", [P, P], BF16, kind="ExternalInput").ap()
    perm_d = nc.dram_tensor("perm", [P, P], BF16, kind="ExternalInput").ap()
    out_d = nc.dram_tensor("out", [4, P, 4, H], F32, kind="ExternalOutput").ap()
    from contextlib import ExitStack

    with tile.TileContext(nc) as tc:
        with ExitStack() as ctx:
            _emit(tc, xT_d, wqk_d, wv_d, mask_d, perm_d, out_d, ctx)
    nc.compile()
    return nc


def _host_inputs(x, Wq, Wk, Wv):
    bf = ml_dtypes.bfloat16
    nb = x.shape[0]
    # xTp[p, c, t] = x[t, 128c + p]
    xT = np.transpose(x, (0, 2, 1))                       # [nb, C, T]
    xTp = np.ascontiguousarray(
        xT.reshape(nb, NCC, P, T).transpose(0, 2, 1, 3)
    ).astype(bf)                                          # [nb, 128, 3, T]
    wqk = np.ascontiguousarray(np.concatenate([Wq, Wk], axis=1)).astype(bf)
    wv = np.ascontiguousarray(Wv).astype(bf)
    # mask[s, t] = 1 where s <= t (transposed-causal, diagonal 128x128 block)
    mask = np.triu(np.ones((P, P), dtype=np.float32)).astype(bf)
    # perm.T @ v rotates partitions by 64: dup[p] = v[(p + 64) % 128]
    perm = np.roll(np.eye(P, dtype=np.float32), 64, axis=0).astype(bf)
    return xTp, wqk, wv, mask, perm


def kernel(x, Wq, Wk, Wv):
    global LAST_RESULT, _PROGRAM
    assert x.shape == (B, T, C), x.shape
    if _PROGRAM is None:
        _PROGRAM = _build_program()
    nc = _PROGRAM

    xTp, wqk, wv, mask, perm = _host_inputs(x, Wq, Wk, Wv)
    in_maps = [
        {"xTp": xTp[b], "wqk": wqk, "wv": wv, "mask": mask, "perm": perm}
        for b in range(B)
    ]
    trace = bool(int(os.environ.get("KERNEL_TRACE", "0")))
    kw = {}
    td = os.environ.get("KERNEL_TRACE_DIR")
    if td:
        kw["tmpdir"] = td
    LAST_RESULT = run_bass_kernel_spmd(
        nc, in_maps, list(range(B)), trace=trace, **kw
    )
    # out dram layout [4, 128, 4, 64] -> [T, H]
    out = np.stack(
        [
            LAST_RESULT.results[b]["out"]
            .transpose(0, 2, 1, 3)
            .reshape(T, H)
            for b in range(B)
        ],
        axis=0,
    )
    return out.astype(np.float32)
